# revision 3
# baseline (speedup 1.0000x reference)
"""Llama4TextExperts MoE expert-parallel kernel for 8 Trainium2 cores.

(v9: reload of the spilled acted half streams as half-tile fp16 DMAs
with 6-deep staging, converted to fp32r by DVE per half — first
reloaded data is ready ~2x sooner at the phase boundary.)

Core e computes expert e: x_e (1024,2048) @ gate_up[e] -> silu(gate)*up
-> @ down[e] -> out_e (1024,2048). All matmuls fp32r (self-loading
stationary weights, zero LDWEIGHTS). Measured-on-HW design choices:
 - 256-token matmul chunks with each PSUM accumulation chain run
   consecutively on one bank (HW-measured ~150 ns/MM vs ~380 for
   512-chunks with bank-interleaved chains).
 - 16 of 32 acted tiles stay SBUF-resident across the phase boundary;
   the rest spill to DRAM as fp16 (4 MB round trip) and reload+convert
   via DVE behind the first phase-2 chains.
 - Weight streams issue from the SP ring (no compute in its queue);
   x / spill / reload / out DMAs ride the Activation ring.


All-fp32r self-loading matmuls (no LDWEIGHTS anywhere), host tile-major
weights, and the phase-1 -> phase-2 handoff engineered so the PE never
waits on DRAM:
 - 16 of 32 acted tiles stay SBUF-resident across the phase boundary.
 - The other 16 spill to DRAM as fp16 (4 MB round trip) and are
   converted back to fp32r by the idle DVE during the first phase-2
   output chains; phase-2 contracts resident tiles first so the reload
   fully hides.
 - Weight streams (w1, w2) issue from the SP ring, whose queue carries
   no compute and never blocks; x / spill / reload / out DMAs ride the
   Activation ring.
 - The first weight pair is split into 4 sub-DMAs so the first matmul
   chain starts ~1 us in; x streams in half-token chunks ordered to
   match the g=0 t-major chain order.
"""

import numpy as np

NUM_EXPERTS = 8
HIDDEN = 2048
INTER = 4096
TOKENS = 8192
T = TOKENS // NUM_EXPERTS  # 1024 tokens per expert/core
TK = HIDDEN // 128  # 16 contraction tiles in phase 1
TI = INTER // 128  # 32 feature tiles of gate/up; contraction tiles in phase 2
TH = HIDDEN // 128  # 16 output feature tiles
NT = T // 512  # default; build_bass(chunk=) overrides
NRES = 16  # acted tiles kept SBUF-resident across the phase boundary
NSP = TI - NRES  # spilled acted tiles


def _split_waits(nc, max_waits=1):
    """The walrus build in this environment rejects instructions carrying
    more than one sync wait. Move excess SyncWaits onto preceding NoOps
    on the same engine (semantically identical: the engine stalls on the
    NoOps first)."""
    import concourse.mybir as mybir

    for fn in nc.m.functions:
        for blk in fn.blocks:
            new_insts = []
            for inst in blk.instructions:
                si = inst.sync_info
                if si is not None and len(si.on_wait) > max_waits:
                    waits = list(si.on_wait)
                    excess, keep = waits[:-max_waits], waits[-max_waits:]
                    for i in range(0, len(excess), max_waits):
                        chunk = excess[i : i + max_waits]
                        new_insts.append(
                            mybir.InstNoOp(
                                name=f"{inst.name}-waitsplit-{i}",
                                ins=[],
                                outs=[],
                                engine=inst.engine,
                                sync_info=mybir.SyncInfo(
                                    on_wait=list(chunk), on_update=[]
                                ),
                            )
                        )
                    si.on_wait = keep
                new_insts.append(inst)
            blk.instructions = new_insts


def build_bass(repeat=1, split_waits=True, chunk=256):
    NT = T // chunk
    CH = chunk
    import contextlib

    import concourse.bass as bass
    import concourse.mybir as mybir
    import concourse.tile as tile

    F32 = mybir.dt.float32
    F32R = mybir.dt.float32r
    F16 = mybir.dt.float16
    Silu = mybir.ActivationFunctionType.Silu

    nc = bass.Bass()
    xT = nc.declare_dram_parameter("xT", [HIDDEN, T], F32R, isOutput=False)
    # host-reordered tile-major: w1[g, p, kk, f] = gate_up[kk*128+p, g*128+f]
    # (g 0..31 = gate blocks, 32..63 = up blocks); w2[h, p, ii, f] = down[ii*128+p, h*128+f]
    w1 = nc.declare_dram_parameter("w1", [2 * TI, 128, TK, 128], F32R, isOutput=False)
    w2 = nc.declare_dram_parameter("w2", [TH, 128, TI, 128], F32R, isOutput=False)
    outT = nc.declare_dram_parameter("outT", [HIDDEN, T], F32, isOutput=True)

    xT_t = xT.rearrange("(kk p) t -> kk p t", p=128)
    outT_t = outT.rearrange("(hh p) t -> hh p t", p=128)

    # fp16 spill of acted tiles NRES..TI-1
    acted_dram = nc.dram_tensor("acted_scratch", [NSP, 128, T], F16)

    with tile.TileContext(nc) as tc:
        rep = tc.For_i(0, repeat, 1) if repeat > 1 else contextlib.nullcontext()
        with rep:
            with tc.tile_pool(name="actres", bufs=1) as arp, \
                 tc.tile_pool(name="w2s", bufs=2) as w2p:
                actres = [
                    arp.tile([128, T], F32R, tag=f"ar{i}", name=f"actres{i}")
                    for i in range(NRES)
                ]

                # ---- Phase 1 ----
                with tc.tile_pool(name="xres", bufs=1) as xp, \
                     tc.tile_pool(name="w1s", bufs=2) as w1p, \
                     tc.tile_pool(name="tmp", bufs=2) as tmpp, \
                     tc.tile_pool(name="astg", bufs=2) as astgp, \
                     tc.tile_pool(name="ps1", bufs=max(1, 4 // NT), space="PSUM") as ps1:
                    xts = [
                        xp.tile([128, T], F32R, tag=f"x{k}", name=f"xres{k}")
                        for k in range(TK)
                    ]
                    # g=0 weight pair in 4 sub-DMAs each on the SP ring so
                    # the first chain starts after ~64 KB, x half-token
                    # chunks on the ACT ring in t-major order
                    wg0 = w1p.tile([128, TK, 128], F32R, tag="wg")
                    wu0 = w1p.tile([128, TK, 128], F32R, tag="wu")
                    nc.scalar.dma_start(out=xts[0][:, :512], in_=xT_t[0][:, :512])
                    for q in range(0, TK, 4):
                        nc.sync.dma_start(
                            out=wg0[:, q : q + 4, :], in_=w1[0][:, q : q + 4, :]
                        )
                    for k in range(1, TK):
                        nc.scalar.dma_start(
                            out=xts[k][:, :512], in_=xT_t[k][:, :512]
                        )
                    for q in range(0, TK, 4):
                        nc.sync.dma_start(
                            out=wu0[:, q : q + 4, :], in_=w1[TI][:, q : q + 4, :]
                        )
                    for k in range(TK):
                        nc.scalar.dma_start(
                            out=xts[k][:, 512:], in_=xT_t[k][:, 512:]
                        )

                    for g in range(TI):
                        if g == 0:
                            wg, wu = wg0, wu0
                        else:
                            wg = w1p.tile([128, TK, 128], F32R, tag="wg")
                            wu = w1p.tile([128, TK, 128], F32R, tag="wu")
                            nc.sync.dma_start(out=wg, in_=w1[g])
                            nc.sync.dma_start(out=wu, in_=w1[TI + g])
                        pg = [
                            ps1.tile([128, CH], F32, tag=f"pg{t}", name=f"psg{g}_{t}")
                            for t in range(NT)
                        ]
                        pu = [
                            ps1.tile([128, CH], F32, tag=f"pu{t}", name=f"psu{g}_{t}")
                            for t in range(NT)
                        ]
                        if g == 0:
                            # t-major: the t=0 chains only need the first
                            # x halves and the first wg0/wu0 sub-DMAs
                            for t in range(NT):
                                ts = slice(t * CH, (t + 1) * CH)
                                for kk in range(TK):
                                    st, sp = kk == 0, kk == TK - 1
                                    nc.tensor.matmul(
                                        pg[t], wg[:, kk, :], xts[kk][:, ts],
                                        start=st, stop=sp,
                                    )
                                for kk in range(TK):
                                    st, sp = kk == 0, kk == TK - 1
                                    nc.tensor.matmul(
                                        pu[t], wu[:, kk, :], xts[kk][:, ts],
                                        start=st, stop=sp,
                                    )
                        else:
                            # consecutive chains: one PSUM bank at a time
                            for t in range(NT):
                                ts = slice(t * CH, (t + 1) * CH)
                                for kk in range(TK):
                                    st, sp = kk == 0, kk == TK - 1
                                    nc.tensor.matmul(
                                        pg[t], wg[:, kk, :], xts[kk][:, ts],
                                        start=st, stop=sp,
                                    )
                                for kk in range(TK):
                                    st, sp = kk == 0, kk == TK - 1
                                    nc.tensor.matmul(
                                        pu[t], wu[:, kk, :], xts[kk][:, ts],
                                        start=st, stop=sp,
                                    )
                        if g < NRES:
                            stg = actres[g]
                        else:
                            stg = astgp.tile([128, T], F16, tag="astg")
                        for t in range(NT):
                            ts = slice(t * CH, (t + 1) * CH)
                            sg = tmpp.tile([128, CH], F32, tag="sg")
                            nc.scalar.activation(sg, pg[t], Silu)
                            nc.vector.tensor_mul(stg[:, ts], sg, pu[t])
                        if g >= NRES:
                            nc.scalar.dma_start(
                                out=acted_dram[g - NRES], in_=stg
                            )

                # ---- Phase 2 ----
                with tc.tile_pool(name="acted2", bufs=1) as act2p, \
                     tc.tile_pool(name="rstg", bufs=6) as rstgp, \
                     tc.tile_pool(name="outs", bufs=2) as outp, \
                     tc.tile_pool(name="ps2", bufs=min(2, 8 // NT), space="PSUM") as ps2:
                    reload_ts = [
                        act2p.tile([128, T], F32R, tag=f"a2{i}", name=f"acted2{i}")
                        for i in range(NSP)
                    ]
                    H2 = T // 2
                    for i in range(NSP):
                        for hf in range(2):
                            hs = slice(hf * H2, (hf + 1) * H2)
                            r16 = rstgp.tile([128, H2], F16, tag="rstg")
                            nc.scalar.dma_start(out=r16, in_=acted_dram[i][:, hs])
                            nc.vector.tensor_copy(reload_ts[i][:, hs], r16)

                    def att(ii):
                        return actres[ii] if ii < NRES else reload_ts[ii - NRES]

                    for h in range(TH):
                        w2h = w2p.tile([128, TI, 128], F32R, tag="w2")
                        nc.sync.dma_start(out=w2h, in_=w2[h])
                        po = [
                            ps2.tile([128, CH], F32, tag=f"po{t}", name=f"pso{h}_{t}")
                            for t in range(NT)
                        ]
                        for t in range(NT):
                            ts = slice(t * CH, (t + 1) * CH)
                            for ii in range(TI):
                                st, sp = ii == 0, ii == TI - 1
                                nc.tensor.matmul(
                                    po[t], w2h[:, ii, :], att(ii)[:, ts],
                                    start=st, stop=sp,
                                )
                        ot = outp.tile([128, T], F32, tag="ot")
                        for t in range(NT):
                            ts = slice(t * CH, (t + 1) * CH)
                            nc.vector.tensor_copy(ot[:, ts], po[t])
                            nc.scalar.dma_start(
                                out=outT_t[h][:, ts], in_=ot[:, ts]
                            )

    if split_waits:
        _split_waits(nc, 1)
    return nc


def make_in_maps(hidden_states, gate_up_proj, down_proj):
    x = np.asarray(hidden_states, dtype=np.float32).reshape(NUM_EXPERTS, T, HIDDEN)
    w1 = np.asarray(gate_up_proj, dtype=np.float32)
    w2 = np.asarray(down_proj, dtype=np.float32)
    in_maps = []
    for e in range(NUM_EXPERTS):
        # (H, 2I) -> (2I/128 g, 128 p, H/128 kk, 128 f) tile-major contiguous
        w1r = w1[e].reshape(TK, 128, 2 * TI, 128).transpose(2, 1, 0, 3)
        # (I, H) -> (H/128 h, 128 p, I/128 ii, 128 f)
        w2r = w2[e].reshape(TI, 128, TH, 128).transpose(2, 1, 0, 3)
        in_maps.append(
            {
                "xT": np.ascontiguousarray(x[e].T),
                "w1": np.ascontiguousarray(w1r),
                "w2": np.ascontiguousarray(w2r),
            }
        )
    return in_maps


def assemble_output(results):
    outs = [results[e]["outT"].T for e in range(NUM_EXPERTS)]
    return np.concatenate(outs, axis=0).astype(np.float32)


def kernel(hidden_states, gate_up_proj, down_proj):
    from concourse.bass_utils import run_bass_kernel_spmd

    nc = build_bass()
    in_maps = make_in_maps(hidden_states, gate_up_proj, down_proj)
    res = run_bass_kernel_spmd(nc, in_maps, list(range(NUM_EXPERTS)))
    return assemble_output(res.results)
